# revision 1
# baseline (speedup 1.0000x reference)
"""Multi-head attention (B=2, S=2048, D=1024, H=16, K=64) on 8 TRN2 cores.

Sharding: core c -> batch b=c//4, head-group g=c%4 (4 heads, 256-wide slice
of Wq/Wk/Wv columns and Wo rows).  Each core computes a partial (2048, 1024)
output; host sums groups of 4 cores and adds bo.

Per-core layout (all transposed so no on-chip transposes are needed):
  - host supplies xT = x[b].T  (D, S), bf16
  - Q^T, K^T computed as [gw_col, S] via lhsT=W chunk, rhs=xT chunk
  - scores^T[j, i] via lhsT=K^T chunk, rhs=Q^T  -> softmax denom via a ones
    column appended to V (V_aug), probs^T = exp(scores^T / 8) with no max
    subtraction (scores are ~N(0,1); exp cannot overflow fp32)
  - O^T = V_aug^T @ probs^T, divided by denom, feeds Wo matmul as stationary

All matmul operands are bf16 (PSUM accumulation stays fp32): bf16 LDWEIGHTS
is 1 cycle/row vs 4 for f32/f32r, and halves SBUF traffic.
"""

import os
import sys
from contextlib import ExitStack

import numpy as np

if "/opt/trn_rl_repo" not in sys.path:
    sys.path.insert(0, "/opt/trn_rl_repo")

import concourse.bass as bass
import concourse.mybir as mybir
import concourse.tile as tile
from concourse import bacc
from concourse.bass import ds, ts
from concourse.bass_utils import run_bass_kernel_spmd

B, S, D = 2, 2048, 1024
H, KS = 16, 64
NCORES = 8
HPC = H // 4          # 4 heads per core
GW = HPC * KS         # 256-wide head-group slice
P = 128
ND = D // P           # 8 contraction chunks over d_model
NM = GW // P          # 2 col chunks of the group slice
NI = 4                # i-groups
IT = S // NI          # 512 rows per i-group
NJ = S // P           # 16 j-chunks
NO = D // 512         # 2 out-col groups for Wo

F32 = mybir.dt.float32
BF16 = mybir.dt.bfloat16
MMDT = BF16
EXP = mybir.ActivationFunctionType.Exp


def _mha_core(tc, out, xT, wq, wk, wv, wo, bq, bk, bv):
    nc = tc.nc
    with ExitStack() as ctx:
        cp = ctx.enter_context(tc.tile_pool(name="const", bufs=1))
        probs_pool = ctx.enter_context(tc.tile_pool(name="probs", bufs=4))
        out_pool = ctx.enter_context(tc.tile_pool(name="outsb", bufs=3))

        # ---- constants / inputs to SBUF ----
        xT_sb = cp.tile([P, ND, S], MMDT)
        for dc in range(ND):
            eng = nc.sync if dc % 2 == 0 else nc.gpsimd
            eng.dma_start(xT_sb[:, dc, :], xT[ts(dc, P), :])
        wq_sb = cp.tile([P, ND, GW], MMDT)
        wk_sb = cp.tile([P, ND, GW], MMDT)
        wv_sb = cp.tile([P, ND, GW], MMDT)
        nc.gpsimd.dma_start(wq_sb[:], wq.rearrange("(nd p) n -> p nd n", p=P))
        nc.gpsimd.dma_start(wk_sb[:], wk.rearrange("(nd p) n -> p nd n", p=P))
        nc.gpsimd.dma_start(wv_sb[:], wv.rearrange("(nd p) n -> p nd n", p=P))
        wo_sb = cp.tile([P, NM, D], MMDT)
        nc.gpsimd.dma_start(wo_sb[:], wo.rearrange("(nm p) n -> p nm n", p=P))
        bq_sb = cp.tile([P, NM], F32)
        bk_sb = cp.tile([P, NM], F32)
        nc.sync.dma_start(bq_sb[:], bq.rearrange("(m p) -> p m", p=P))
        nc.sync.dma_start(bk_sb[:], bk.rearrange("(m p) -> p m", p=P))
        bv_bc = cp.tile([P, GW], F32)
        nc.sync.dma_start(bv_bc[:], bv.partition_broadcast(P))
        ones_f32 = cp.tile([P, HPC * NJ], F32)
        nc.vector.memset(ones_f32[:], 1.0)
        ones64 = cp.tile([1, KS], MMDT)
        nc.scalar.copy(ones64[:], ones_f32[ds(0, 1), 0:KS])

        QT = cp.tile([P, NM, S], MMDT)
        KT = cp.tile([P, NM, S], MMDT)
        OT = cp.tile([P, NM, S], MMDT)
        # V_aug[:, h, jt, 0:64] = V rows, [:, h, jt, 64] = 1.0 (denominator col)
        V_aug = cp.tile([P, HPC, NJ, KS + 1], MMDT)
        nc.scalar.copy(
            V_aug[:, :, :, ds(KS, 1)].rearrange("p h j o -> p (h j o)"),
            ones_f32[:])

        # ---- projections (own PSUM scope, closed before attention) ----
        with tc.tile_pool(name="ps_acc", bufs=4, space="PSUM") as ps_acc:
            # Q^T / K^T: [gw_col, S]
            for m in range(NM):
                for ig in range(NI):
                    qt_ps = ps_acc.tile([P, IT], F32, tag="acc")
                    kt_ps = ps_acc.tile([P, IT], F32, tag="acc")
                    for dc in range(ND):
                        nc.tensor.matmul(
                            qt_ps[:],
                            wq_sb[:, dc, ts(m, P)],
                            xT_sb[:, dc, ts(ig, IT)],
                            start=(dc == 0), stop=(dc == ND - 1),
                        )
                    for dc in range(ND):
                        nc.tensor.matmul(
                            kt_ps[:],
                            wk_sb[:, dc, ts(m, P)],
                            xT_sb[:, dc, ts(ig, IT)],
                            start=(dc == 0), stop=(dc == ND - 1),
                        )
                    nc.vector.tensor_scalar_add(
                        QT[:, m, ts(ig, IT)], qt_ps[:], bq_sb[:, ds(m, 1)])
                    nc.vector.tensor_scalar_add(
                        KT[:, m, ts(ig, IT)], kt_ps[:], bk_sb[:, ds(m, 1)])

            # V (natural layout) + bias + ones col
            for jt in range(NJ):
                v_ps = ps_acc.tile([P, IT], F32, tag="acc")
                for dc in range(ND):
                    nc.tensor.matmul(
                        v_ps[:, 0:GW],
                        xT_sb[:, dc, ts(jt, P)],
                        wv_sb[:, dc, :],
                        start=(dc == 0), stop=(dc == ND - 1),
                    )
                nc.vector.tensor_add(
                    V_aug[:, :, jt, 0:KS],
                    v_ps[:, 0:GW].rearrange("p (h k) -> p h k", h=HPC),
                    bv_bc[:].rearrange("p (h k) -> p h k", h=HPC),
                )

        # ---- attention + output projection, i-group major ----
        with tc.tile_pool(name="ps_s", bufs=4, space="PSUM") as ps_s, \
             tc.tile_pool(name="ps_o", bufs=2, space="PSUM") as ps_o, \
             tc.tile_pool(name="ps_w", bufs=1, space="PSUM") as ps_w, \
             tc.tile_pool(name="ps_c", bufs=1, space="PSUM") as ps_c:
            for ig in range(NI):
                for h in range(HPC):
                    po = 64 * (h % 2)   # partition offset of head h in its NM tile
                    m = h // 2
                    o_ps = ps_o.tile([KS + 1, IT], F32)
                    for jc in range(NJ):
                        s_ps = ps_s.tile([P, IT], F32)
                        nc.tensor.matmul(
                            s_ps[:],
                            KT[ds(po, KS), m, ts(jc, P)],
                            QT[ds(po, KS), m, ts(ig, IT)],
                            start=True, stop=True,
                        )
                        pt = probs_pool.tile([P, IT], MMDT)
                        nc.scalar.activation(pt[:], s_ps[:], EXP, scale=0.125)
                        nc.tensor.matmul(
                            o_ps[:],
                            V_aug[:, h, jc, :],
                            pt[:],
                            start=(jc == 0), stop=(jc == NJ - 1),
                        )
                    den_sb = out_pool.tile([1, IT], F32)
                    nc.vector.tensor_copy(den_sb[:], o_ps[ds(KS, 1), :])
                    recip32 = out_pool.tile([1, IT], F32)
                    nc.vector.reciprocal_approx_fast(recip32[:], den_sb[:])
                    recip = out_pool.tile([1, IT], MMDT)
                    nc.gpsimd.tensor_copy(recip[:], recip32[:])
                    bc_ps = ps_c.tile([KS, IT], F32)
                    nc.tensor.matmul(bc_ps[:], ones64[:], recip[:],
                                     start=True, stop=True)
                    bc_sb = out_pool.tile([KS, IT], F32)
                    nc.vector.tensor_copy(bc_sb[:], bc_ps[:])
                    nc.vector.tensor_mul(
                        OT[ds(po, KS), m, ts(ig, IT)], o_ps[ds(0, KS), :],
                        bc_sb[:])

                # Wo partial for the 4 row-tiles of this i-group
                for itl in range(NI):
                    it = ig * NI + itl
                    for ncol in range(NO):
                        w_ps = ps_w.tile([P, 512], F32)
                        for hc in range(NM):
                            nc.tensor.matmul(
                                w_ps[:],
                                OT[:, hc, ts(it, P)],
                                wo_sb[:, hc, ts(ncol, 512)],
                                start=(hc == 0), stop=(hc == NM - 1),
                            )
                        o_sb = out_pool.tile([P, 512], F32)
                        nc.vector.tensor_copy(o_sb[:], w_ps[:])
                        eng = nc.sync if (it + ncol) % 2 == 0 else nc.gpsimd
                        eng.dma_start(out[ts(it, P), ts(ncol, 512)], o_sb[:])


def _build_program():
    nc = bacc.Bacc("TRN2", target_bir_lowering=False, debug=False,
                   num_devices=NCORES)
    xT = nc.dram_tensor("xT", (D, S), MMDT, kind="ExternalInput").ap()
    wq = nc.dram_tensor("wq", (D, GW), MMDT, kind="ExternalInput").ap()
    wk = nc.dram_tensor("wk", (D, GW), MMDT, kind="ExternalInput").ap()
    wv = nc.dram_tensor("wv", (D, GW), MMDT, kind="ExternalInput").ap()
    wo = nc.dram_tensor("wo", (GW, D), MMDT, kind="ExternalInput").ap()
    bq = nc.dram_tensor("bq", (GW,), F32, kind="ExternalInput").ap()
    bk = nc.dram_tensor("bk", (GW,), F32, kind="ExternalInput").ap()
    bv = nc.dram_tensor("bv", (GW,), F32, kind="ExternalInput").ap()
    out = nc.dram_tensor("out", (S, D), F32, kind="ExternalOutput").ap()
    with tile.TileContext(nc) as tc:
        _mha_core(tc, out, xT, wq, wk, wv, wo, bq, bk, bv)
    nc.compile()
    return nc


_program = None


def _get_program():
    global _program
    if _program is None:
        _program = _build_program()
    return _program


def make_in_maps(x, Wq, bq, Wk, bk, Wv, bv, Wo, bo):
    in_maps = []
    f = np.float32
    bf = mybir.dt.np(MMDT)
    for c in range(NCORES):
        b, g = divmod(c, 4)
        sl = slice(g * GW, (g + 1) * GW)
        in_maps.append({
            "xT": np.ascontiguousarray(x[b].T).astype(bf),
            "wq": np.ascontiguousarray(Wq[:, sl]).astype(bf),
            "wk": np.ascontiguousarray(Wk[:, sl]).astype(bf),
            "wv": np.ascontiguousarray(Wv[:, sl]).astype(bf),
            "wo": np.ascontiguousarray(Wo[sl, :]).astype(bf),
            "bq": np.ascontiguousarray(bq[sl], dtype=f),
            "bk": np.ascontiguousarray(bk[sl], dtype=f),
            "bv": np.ascontiguousarray(bv[sl], dtype=f),
        })
    return in_maps


def run(inputs, trace=False, tmpdir=None):
    nc = _get_program()
    in_maps = make_in_maps(**inputs)
    res = run_bass_kernel_spmd(nc, in_maps, core_ids=list(range(NCORES)),
                               trace=trace, tmpdir=tmpdir)
    bo = inputs["bo"].astype(np.float32)
    parts = [res.results[c]["out"] for c in range(NCORES)]
    y = np.stack(
        [parts[4 * b] + parts[4 * b + 1] + parts[4 * b + 2] + parts[4 * b + 3] + bo
         for b in range(B)], axis=0)
    return y.astype(np.float32), res


def kernel(**inputs):
    y, _ = run(inputs, trace=False)
    return y



# revision 4
# speedup vs baseline: 185.1416x; 185.1416x over previous
"""Multi-head attention (B=2, S=2048, D=1024, H=16, K=64) on 8 TRN2 cores.

Sharding: core c -> batch b=c//4, head-group g=c%4 (4 heads, 256-wide slice
of Wq/Wk/Wv columns and Wo rows).  Each core computes a partial (2048, 1024)
output in bf16; host sums groups of 4 cores in f32 and adds bo.

Per-core layout (all transposed so no on-chip transposes are needed):
  - host supplies xT = x[b].T  (D, S), bf16
  - Q^T, K^T computed as [gw_col, S] via lhsT=W chunk, rhs=xT chunk
  - scores^T[j, i] via lhsT=K^T chunk, rhs=Q^T; four 512-col j-chunks land in
    one 4-bank PSUM tile so a single ScalarE Exp covers 2048 elements
    (amortizes the ~352-cycle ACT overhead; ACT is the bottleneck engine)
  - softmax denominator via a ones column appended to V (V_aug); probs are
    exp(scores/8) with no max subtraction (scores ~N(0,1), no overflow)
  - O^T = V_aug^T @ probs^T; division by the denominator happens on VectorE
    with head-PAIR batching: one reciprocal + one broadcast-matmul per pair,
    emitted AFTER the next head's matmuls so the PE never stalls on the
    reciprocal chain (the v1 kernel lost 2x to HAM re-throttling here)

All matmul operands are bf16 (PSUM accumulation stays fp32).
"""

import os
import sys
from contextlib import ExitStack

import numpy as np

if "/opt/trn_rl_repo" not in sys.path:
    sys.path.insert(0, "/opt/trn_rl_repo")

import concourse.bass as bass
import concourse.mybir as mybir
import concourse.tile as tile
from concourse import bacc
from concourse.bass import ds, ts
from concourse.bass_utils import run_bass_kernel_spmd

B, S, D = 2, 2048, 1024
H, KS = 16, 64
NCORES = 8
HPC = H // 4          # 4 heads per core
GW = HPC * KS         # 256-wide head-group slice
P = 128
ND = D // P           # 8 contraction chunks over d_model
NM = GW // P          # 2 col chunks of the group slice
NI = 4                # i-groups
IT = S // NI          # 512 rows per i-group
NJ = S // P           # 16 j-chunks
NJJ = 4               # j-chunks per Exp batch (4 x 512 = one [128,2048] ACT)
NO = D // 512         # 2 out-col groups for Wo

F32 = mybir.dt.float32
BF16 = mybir.dt.bfloat16
MMDT = BF16
EXP = mybir.ActivationFunctionType.Exp


def _mha_core(tc, out, xT, wq, wk, wv, wo, bq, bk, bv):
    nc = tc.nc
    with ExitStack() as ctx:
        cp = ctx.enter_context(tc.tile_pool(name="const", bufs=1))
        probs_pool = ctx.enter_context(tc.tile_pool(name="probs", bufs=2))
        out_pool = ctx.enter_context(tc.tile_pool(name="outsb", bufs=3))
        den_pool = ctx.enter_context(tc.tile_pool(name="den", bufs=2))

        # ---- ACT table preload: tiny exp before anything else on ScalarE ----
        warm = cp.tile([1, 16], F32)
        nc.vector.memset(warm[:], 0.0)
        nc.scalar.activation(warm[:], warm[:], EXP)

        # ---- inputs to SBUF; sync ring carries what's needed first ----
        wk_sb = cp.tile([P, ND, GW], MMDT)
        wq_sb = cp.tile([P, ND, GW], MMDT)
        nc.sync.dma_start(wk_sb[:], wk.rearrange("(nd p) n -> p nd n", p=P))
        nc.sync.dma_start(wq_sb[:], wq.rearrange("(nd p) n -> p nd n", p=P))
        xT_sb = cp.tile([P, ND, S], MMDT)
        nc.sync.dma_start(
            xT_sb[:, :, 0:1024],
            xT[:, 0:1024].rearrange("(nd p) s -> p nd s", p=P))
        nc.sync.dma_start(
            xT_sb[:, :, 1024:2048],
            xT[:, 1024:2048].rearrange("(nd p) s -> p nd s", p=P))
        wv_sb = cp.tile([P, ND, GW], MMDT)
        wo_sb = cp.tile([P, NM, D], MMDT)
        nc.scalar.dma_start(wv_sb[:], wv.rearrange("(nd p) n -> p nd n", p=P))
        nc.scalar.dma_start(wo_sb[:], wo.rearrange("(nm p) n -> p nm n", p=P))
        bq_sb = cp.tile([P, NM], F32)
        bk_sb = cp.tile([P, NM], F32)
        nc.scalar.dma_start(bq_sb[:], bq.rearrange("(m p) -> p m", p=P))
        nc.scalar.dma_start(bk_sb[:], bk.rearrange("(m p) -> p m", p=P))
        bv_bc = cp.tile([P, GW], F32)
        nc.scalar.dma_start(bv_bc[:], bv.partition_broadcast(P))

        QT = cp.tile([P, NM, S], MMDT)
        KT = cp.tile([P, NM, S], MMDT)
        OT = cp.tile([P, NM, S], MMDT)
        # V_aug[:, h, jt, 0:64] = V rows, [:, h, jt, 64] = 1.0 (denominator col)
        V_aug = cp.tile([P, HPC, NJ, KS + 1], MMDT)
        nc.vector.memset(
            V_aug[:, :, :, ds(KS, 1)].rearrange("p h j o -> p (h j o)"), 1.0)

        # ones row for the denominator broadcast matmul (1/den -> 64 rows)
        ones64 = cp.tile([1, KS], MMDT)
        nc.vector.memset(ones64[:], 1.0)

        # ---- projections (own PSUM scope, closed before attention) ----
        with tc.tile_pool(name="ps_acc", bufs=4, space="PSUM") as ps_acc:
            # K^T: [gw_col, S]
            for m in range(NM):
                for ig in range(NI):
                    kt_ps = ps_acc.tile([P, IT], F32, tag="acc")
                    for dc in range(ND):
                        nc.tensor.matmul(
                            kt_ps[:],
                            wk_sb[:, dc, ts(m, P)],
                            xT_sb[:, dc, ts(ig, IT)],
                            start=(dc == 0), stop=(dc == ND - 1),
                        )
                    nc.vector.tensor_scalar_add(
                        KT[:, m, ts(ig, IT)], kt_ps[:], bk_sb[:, ds(m, 1)])

            # V (natural layout) + bias + ones col
            for jt in range(NJ):
                v_ps = ps_acc.tile([P, IT], F32, tag="acc")
                for dc in range(ND):
                    nc.tensor.matmul(
                        v_ps[:, 0:GW],
                        xT_sb[:, dc, ts(jt, P)],
                        wv_sb[:, dc, :],
                        start=(dc == 0), stop=(dc == ND - 1),
                    )
                nc.vector.tensor_add(
                    V_aug[:, :, jt, 0:KS],
                    v_ps[:, 0:GW].rearrange("p (h k) -> p h k", h=HPC),
                    bv_bc[:].rearrange("p (h k) -> p h k", h=HPC),
                )

            # Q^T
            for ig in range(NI):
                for m in range(NM):
                    qt_ps = ps_acc.tile([P, IT], F32, tag="acc")
                    for dc in range(ND):
                        nc.tensor.matmul(
                            qt_ps[:],
                            wq_sb[:, dc, ts(m, P)],
                            xT_sb[:, dc, ts(ig, IT)],
                            start=(dc == 0), stop=(dc == ND - 1),
                        )
                    nc.vector.tensor_scalar_add(
                        QT[:, m, ts(ig, IT)], qt_ps[:], bq_sb[:, ds(m, 1)])

        # ---- attention + output projection, i-group major ----
        # PSUM budget (8 banks): s4 = 4, o_ps x2 = 2, bc = 1, w = 1
        with tc.tile_pool(name="ps_s", bufs=1, space="PSUM") as ps_s, \
             tc.tile_pool(name="ps_o", bufs=2, space="PSUM") as ps_o, \
             tc.tile_pool(name="ps_w", bufs=1, space="PSUM") as ps_w, \
             tc.tile_pool(name="ps_c", bufs=1, space="PSUM") as ps_c:

            def den_chain(ig, m, o_even, o_odd):
                """Normalize heads 2m, 2m+1 of i-group ig (one reciprocal
                for the pair; broadcast matmuls target the two column
                groups of one PSUM bank)."""
                den2 = den_pool.tile([1, 2, IT], F32)
                nc.vector.tensor_copy(den2[:, 0, :], o_even[ds(KS, 1), :])
                nc.vector.tensor_copy(den2[:, 1, :], o_odd[ds(KS, 1), :])
                recip2 = den_pool.tile([1, 2, IT], F32)
                nc.vector.reciprocal_approx_fast(
                    recip2[:].rearrange("p a b -> p (a b)"),
                    den2[:].rearrange("p a b -> p (a b)"))
                recip2b = den_pool.tile([1, 2, IT], MMDT)
                nc.vector.tensor_copy(
                    recip2b[:].rearrange("p a b -> p (a b)"),
                    recip2[:].rearrange("p a b -> p (a b)"))
                bc_ps = ps_c.tile([P, IT], F32)
                nc.tensor.matmul(bc_ps[ds(0, KS), :], ones64[:],
                                 recip2b[:, 0, :], start=True, stop=True)
                nc.tensor.matmul(bc_ps[ds(KS, KS), :], ones64[:],
                                 recip2b[:, 1, :], start=True, stop=True,
                                 tile_position=(0, KS))
                bc_sb = den_pool.tile([P, IT], F32)
                nc.vector.tensor_copy(bc_sb[:], bc_ps[:])
                nc.vector.tensor_mul(
                    OT[ds(0, KS), m, ts(ig, IT)], o_even[ds(0, KS), :],
                    bc_sb[ds(0, KS), :])
                nc.vector.tensor_mul(
                    OT[ds(KS, KS), m, ts(ig, IT)], o_odd[ds(0, KS), :],
                    bc_sb[ds(KS, KS), :])

            for ig in range(NI):
                o_pending = None
                for h in range(HPC):
                    po = KS * (h % 2)   # partition offset of head h in NM tile
                    m = h // 2
                    o_ps = ps_o.tile([KS + 1, IT], F32)
                    for jj in range(NJ // NJJ):
                        s4 = ps_s.tile([P, NJJ, IT], F32)
                        for q in range(NJJ):
                            jc = jj * NJJ + q
                            nc.tensor.matmul(
                                s4[:, q, :],
                                KT[ds(po, KS), m, ts(jc, P)],
                                QT[ds(po, KS), m, ts(ig, IT)],
                                start=True, stop=True,
                            )
                        pt = probs_pool.tile([P, NJJ, IT], MMDT)
                        nc.scalar.activation(
                            pt[:].rearrange("p a b -> p (a b)"),
                            s4[:].rearrange("p a b -> p (a b)"),
                            EXP, scale=0.125)
                        for q in range(NJJ):
                            jc = jj * NJJ + q
                            nc.tensor.matmul(
                                o_ps[:],
                                V_aug[:, h, jc, :],
                                pt[:, q, :],
                                start=(jj == 0 and q == 0),
                                stop=(jj == NJ // NJJ - 1 and q == NJJ - 1),
                            )
                    if h % 2 == 0:
                        o_pending = o_ps
                    else:
                        # emitted after head h's matmuls are queued, so the
                        # PE works on head h while VectorE runs the chain
                        den_chain(ig, m, o_pending, o_ps)
                        o_pending = None

                # Wo partial for the 4 row-tiles of this i-group
                for itl in range(NI):
                    it = ig * NI + itl
                    for ncol in range(NO):
                        w_ps = ps_w.tile([P, 512], F32)
                        for hc in range(NM):
                            nc.tensor.matmul(
                                w_ps[:],
                                OT[:, hc, ts(it, P)],
                                wo_sb[:, hc, ts(ncol, 512)],
                                start=(hc == 0), stop=(hc == NM - 1),
                            )
                        o_sb = out_pool.tile([P, 512], MMDT)
                        nc.vector.tensor_copy(o_sb[:], w_ps[:])
                        eng = nc.sync if (it + ncol) % 2 == 0 else nc.scalar
                        eng.dma_start(out[ts(it, P), ts(ncol, 512)], o_sb[:])


def _build_program():
    nc = bacc.Bacc("TRN2", target_bir_lowering=False, debug=False,
                   num_devices=NCORES)
    xT = nc.dram_tensor("xT", (D, S), MMDT, kind="ExternalInput").ap()
    wq = nc.dram_tensor("wq", (D, GW), MMDT, kind="ExternalInput").ap()
    wk = nc.dram_tensor("wk", (D, GW), MMDT, kind="ExternalInput").ap()
    wv = nc.dram_tensor("wv", (D, GW), MMDT, kind="ExternalInput").ap()
    wo = nc.dram_tensor("wo", (GW, D), MMDT, kind="ExternalInput").ap()
    bq = nc.dram_tensor("bq", (GW,), F32, kind="ExternalInput").ap()
    bk = nc.dram_tensor("bk", (GW,), F32, kind="ExternalInput").ap()
    bv = nc.dram_tensor("bv", (GW,), F32, kind="ExternalInput").ap()
    out = nc.dram_tensor("out", (S, D), MMDT, kind="ExternalOutput").ap()
    with tile.TileContext(nc) as tc:
        _mha_core(tc, out, xT, wq, wk, wv, wo, bq, bk, bv)
    nc.compile()
    return nc


_program = None


def _get_program():
    global _program
    if _program is None:
        _program = _build_program()
    return _program


def make_in_maps(x, Wq, bq, Wk, bk, Wv, bv, Wo, bo):
    in_maps = []
    f = np.float32
    bf = mybir.dt.np(MMDT)
    for c in range(NCORES):
        b, g = divmod(c, 4)
        sl = slice(g * GW, (g + 1) * GW)
        in_maps.append({
            "xT": np.ascontiguousarray(x[b].T).astype(bf),
            "wq": np.ascontiguousarray(Wq[:, sl]).astype(bf),
            "wk": np.ascontiguousarray(Wk[:, sl]).astype(bf),
            "wv": np.ascontiguousarray(Wv[:, sl]).astype(bf),
            "wo": np.ascontiguousarray(Wo[sl, :]).astype(bf),
            "bq": np.ascontiguousarray(bq[sl], dtype=f),
            "bk": np.ascontiguousarray(bk[sl], dtype=f),
            "bv": np.ascontiguousarray(bv[sl], dtype=f),
        })
    return in_maps


def run(inputs, trace=False, tmpdir=None, **kw):
    nc = _get_program()
    in_maps = make_in_maps(**inputs)
    res = run_bass_kernel_spmd(nc, in_maps, core_ids=list(range(NCORES)),
                               trace=trace, tmpdir=tmpdir, **kw)
    bo = inputs["bo"].astype(np.float32)
    parts = [np.asarray(res.results[c]["out"], dtype=np.float32)
             for c in range(NCORES)]
    y = np.stack(
        [parts[4 * b] + parts[4 * b + 1] + parts[4 * b + 2] + parts[4 * b + 3] + bo
         for b in range(B)], axis=0)
    return y.astype(np.float32), res


def kernel(**inputs):
    y, _ = run(inputs, trace=False)
    return y


# revision 8
# speedup vs baseline: 298.7277x; 1.6135x over previous
"""Multi-head attention (B=2, S=2048, D=1024, H=16, K=64) on 8 TRN2 cores.

Sharding: core c -> batch b=c//4, head-group g=c%4 (4 heads, 256-wide slice
of Wq/Wk/Wv columns and Wo rows).  Each core computes a partial (2048, 1024)
output in bf16; host sums groups of 4 cores in f32 and adds bo.

Per-core layout (all transposed so no on-chip transposes are needed):
  - host supplies xT = x[b].T  (D, S), bf16
  - Q^T, K^T computed as [gw_col, S] via lhsT=W chunk, rhs=xT chunk
  - scores^T[j, i] via lhsT=K^T chunk, rhs=Q^T into double-buffered 2-bank
    PSUM tiles; one ScalarE Exp covers 1024 elements
  - softmax denominator via a ones column appended to V (V_aug); probs are
    exp(scores/8) with no max subtraction (scores ~N(0,1), no overflow)
  - O^T = V_aug^T @ probs^T; division by the denominator happens on VectorE
    with head-pair batching (one reciprocal per pair, broadcast matmuls to
    the two column groups of one PSUM bank)

Scheduling: the attention inner loop alone cannot keep TensorE busy (the
Exp on ScalarE is the per-iteration rate limiter), and PE micro-idles make
the HAM clock-gate re-throttle the array to 1.2 GHz.  So the Wo matmuls of
the previous i-group and the Q^T projection of the next i-group are pumped
into the PE queue as filler work between score groups, keeping the PE
saturated and warm.  AV matmuls are emitted one score-group behind so the
PE never head-of-line blocks on the Exp.

All matmul operands are bf16 (PSUM accumulation stays fp32).
"""

import os
import sys
from contextlib import ExitStack

import numpy as np

if "/opt/trn_rl_repo" not in sys.path:
    sys.path.insert(0, "/opt/trn_rl_repo")

import concourse.bass as bass
import concourse.mybir as mybir
import concourse.tile as tile
from concourse import bacc
from concourse.bass import ds, ts
from concourse.bass_utils import run_bass_kernel_spmd

B, S, D = 2, 2048, 1024
H, KS = 16, 64
NCORES = 8
HPC = H // 4          # 4 heads per core
GW = HPC * KS         # 256-wide head-group slice
P = 128
ND = D // P           # 8 contraction chunks over d_model
NM = GW // P          # 2 col chunks of the group slice
NI = 4                # i-groups
IT = S // NI          # 512 rows per i-group
NJ = S // P           # 16 j-chunks
NJJ = 2               # j-chunks per Exp batch ([128,1024] ACT, 2 PSUM banks)
NG = NJ // NJJ        # score groups per head
NO = D // 512         # 2 out-col groups for Wo

F32 = mybir.dt.float32
BF16 = mybir.dt.bfloat16
MMDT = BF16
EXP = mybir.ActivationFunctionType.Exp


def _mha_core(tc, out, xT, wq, wk, wv, wo, bq, bk, bv):
    nc = tc.nc
    with ExitStack() as ctx:
        cp = ctx.enter_context(tc.tile_pool(name="const", bufs=1))
        probs_pool = ctx.enter_context(tc.tile_pool(name="probs", bufs=3))
        out_pool = ctx.enter_context(tc.tile_pool(name="outsb", bufs=3))
        den_pool = ctx.enter_context(tc.tile_pool(name="den", bufs=2))

        # ---- ACT table preload: tiny exp before anything else on ScalarE ----
        warm = cp.tile([1, 16], F32)
        nc.vector.memset(warm[:], 0.0)
        nc.scalar.activation(warm[:], warm[:], EXP)

        # ---- inputs to SBUF; sync ring carries what's needed first ----
        wk_sb = cp.tile([P, ND, GW], MMDT)
        nc.sync.dma_start(wk_sb[:], wk.rearrange("(nd p) n -> p nd n", p=P))
        xT_sb = cp.tile([P, ND, S], MMDT)
        for c4 in range(4):
            nc.sync.dma_start(
                xT_sb[:, :, ts(c4, IT)],
                xT[:, ts(c4, IT)].rearrange("(nd p) s -> p nd s", p=P))
        wq_sb = cp.tile([P, ND, GW], MMDT)
        nc.sync.dma_start(wq_sb[:], wq.rearrange("(nd p) n -> p nd n", p=P))
        wv_sb = cp.tile([P, ND, GW], MMDT)
        wo_sb = cp.tile([P, NM, D], MMDT)
        nc.scalar.dma_start(wv_sb[:], wv.rearrange("(nd p) n -> p nd n", p=P))
        nc.scalar.dma_start(wo_sb[:], wo.rearrange("(nm p) n -> p nm n", p=P))
        bq_sb = cp.tile([P, NM], F32)
        bk_sb = cp.tile([P, NM], F32)
        nc.scalar.dma_start(bq_sb[:], bq.rearrange("(m p) -> p m", p=P))
        nc.scalar.dma_start(bk_sb[:], bk.rearrange("(m p) -> p m", p=P))
        bv_bc = cp.tile([P, GW], F32)
        nc.scalar.dma_start(bv_bc[:], bv.partition_broadcast(P))

        QT = cp.tile([P, NM, S], MMDT)
        KT = cp.tile([P, NM, S], MMDT)
        OT = cp.tile([P, NM, S], MMDT)
        # V_aug[:, h, jt, 0:64] = V rows, [:, h, jt, 64] = 1.0 (denominator col)
        V_aug = cp.tile([P, HPC, NJ, KS + 1], MMDT)
        nc.vector.memset(
            V_aug[:, :, :, ds(KS, 1)].rearrange("p h j o -> p (h j o)"), 1.0)

        # ones row for the denominator broadcast matmul (1/den -> 64 rows)
        ones64 = cp.tile([1, KS], MMDT)
        nc.vector.memset(ones64[:], 1.0)

        # ---- projections: K^T (ig-major, chasing the xT DMAs), V, Q^T(0) ----
        with tc.tile_pool(name="ps_acc", bufs=4, space="PSUM") as ps_acc:
            for ig in range(NI):
                for m in range(NM):
                    kt_ps = ps_acc.tile([P, IT], F32, tag="acc")
                    for dc in range(ND):
                        nc.tensor.matmul(
                            kt_ps[:],
                            wk_sb[:, dc, ts(m, P)],
                            xT_sb[:, dc, ts(ig, IT)],
                            start=(dc == 0), stop=(dc == ND - 1),
                        )
                    nc.vector.tensor_scalar_add(
                        KT[:, m, ts(ig, IT)], kt_ps[:], bk_sb[:, ds(m, 1)])

            for jt in range(NJ):
                v_ps = ps_acc.tile([P, IT], F32, tag="acc")
                for dc in range(ND):
                    nc.tensor.matmul(
                        v_ps[:, 0:GW],
                        xT_sb[:, dc, ts(jt, P)],
                        wv_sb[:, dc, :],
                        start=(dc == 0), stop=(dc == ND - 1),
                    )
                nc.vector.tensor_add(
                    V_aug[:, :, jt, 0:KS],
                    v_ps[:, 0:GW].rearrange("p (h k) -> p h k", h=HPC),
                    bv_bc[:].rearrange("p (h k) -> p h k", h=HPC),
                )

            for m in range(NM):
                qt_ps = ps_acc.tile([P, IT], F32, tag="acc")
                for dc in range(ND):
                    nc.tensor.matmul(
                        qt_ps[:],
                        wq_sb[:, dc, ts(m, P)],
                        xT_sb[:, dc, ts(0, IT)],
                        start=(dc == 0), stop=(dc == ND - 1),
                    )
                nc.vector.tensor_scalar_add(
                    QT[:, m, ts(0, IT)], qt_ps[:], bq_sb[:, ds(m, 1)])

        # ---- attention + interleaved Wo / Q^T-projection filler ----
        # PSUM budget: s2 = 2x2, o_ps = 2, misc(w/qt/bc) = 1 shared FIFO
        with tc.tile_pool(name="ps_s", bufs=2, space="PSUM") as ps_s, \
             tc.tile_pool(name="ps_o", bufs=2, space="PSUM") as ps_o, \
             tc.tile_pool(name="ps_m", bufs=1, space="PSUM") as ps_m:
            ps_c = ps_m

            def wo_unit(it, ncol):
                def emit():
                    w_ps = ps_m.tile([P, 512], F32, tag="m")
                    for hc in range(NM):
                        nc.tensor.matmul(
                            w_ps[:],
                            OT[:, hc, ts(it, P)],
                            wo_sb[:, hc, ts(ncol, 512)],
                            start=(hc == 0), stop=(hc == NM - 1),
                        )
                    o_sb = out_pool.tile([P, 512], MMDT)
                    nc.vector.tensor_copy(o_sb[:], w_ps[:])
                    eng = nc.sync if (it + ncol) % 2 == 0 else nc.scalar
                    eng.dma_start(out[ts(it, P), ts(ncol, 512)], o_sb[:])
                return emit

            def qt_units(g, m):
                """Q^T projection of i-group g, col chunk m, as 4 filler
                units of 2 accumulating matmuls each."""
                state = {}

                def unit(k):
                    def emit():
                        if k == 0:
                            state["ps"] = ps_m.tile([P, IT], F32,
                                                    name="qt_fill_ps", tag="m")
                        qt_ps = state["ps"]
                        for dc in (2 * k, 2 * k + 1):
                            nc.tensor.matmul(
                                qt_ps[:],
                                wq_sb[:, dc, ts(m, P)],
                                xT_sb[:, dc, ts(g, IT)],
                                start=(dc == 0), stop=(dc == ND - 1),
                            )
                        if k == 3:
                            nc.vector.tensor_scalar_add(
                                QT[:, m, ts(g, IT)], qt_ps[:],
                                bq_sb[:, ds(m, 1)])
                    return emit
                return [unit(k) for k in range(4)]

            def den_chain(ig, m, o_even, o_odd):
                """Normalize heads 2m, 2m+1 of i-group ig."""
                den2 = den_pool.tile([1, 2, IT], F32)
                nc.vector.tensor_copy(den2[:, 0, :], o_even[ds(KS, 1), :])
                nc.vector.tensor_copy(den2[:, 1, :], o_odd[ds(KS, 1), :])
                recip2 = den_pool.tile([1, 2, IT], F32)
                nc.vector.reciprocal_approx_fast(
                    recip2[:].rearrange("p a b -> p (a b)"),
                    den2[:].rearrange("p a b -> p (a b)"))
                recip2b = den_pool.tile([1, 2, IT], MMDT)
                nc.vector.tensor_copy(
                    recip2b[:].rearrange("p a b -> p (a b)"),
                    recip2[:].rearrange("p a b -> p (a b)"))
                bc_ps = ps_c.tile([P, IT], F32, tag="m")
                nc.tensor.matmul(bc_ps[ds(0, KS), :], ones64[:],
                                 recip2b[:, 0, :], start=True, stop=True)
                nc.tensor.matmul(bc_ps[ds(KS, KS), :], ones64[:],
                                 recip2b[:, 1, :], start=True, stop=True,
                                 tile_position=(0, KS))
                bc_sb = den_pool.tile([P, IT], F32)
                nc.vector.tensor_copy(bc_sb[:], bc_ps[:])
                nc.vector.tensor_mul(
                    OT[ds(0, KS), m, ts(ig, IT)], o_even[ds(0, KS), :],
                    bc_sb[ds(0, KS), :])
                nc.vector.tensor_mul(
                    OT[ds(KS, KS), m, ts(ig, IT)], o_odd[ds(0, KS), :],
                    bc_sb[ds(KS, KS), :])

            for ig in range(NI):
                # filler work for this i-group's score loop:
                #   heads 0/1 slots <- Wo of ig-1 (or QT of ig+1 when ig==0)
                #   heads 2/3 slots <- QT of ig+1
                fill01, fill23 = [], []
                if ig == 0:
                    fill01 = qt_units(1, 0) + qt_units(1, 1)
                else:
                    fill01 = [wo_unit(4 * (ig - 1) + itl, ncol)
                              for itl in range(NI) for ncol in range(NO)]
                    if ig < NI - 1:
                        fill23 = qt_units(ig + 1, 0) + qt_units(ig + 1, 1)

                o_pending = None
                for h in range(HPC):
                    po = KS * (h % 2)
                    m = h // 2
                    fillers = fill01 if h < 2 else fill23
                    o_ps = ps_o.tile([KS + 1, IT], F32)
                    prev = None  # (pt tile, jj) awaiting AV emission
                    for jj in range(NG):
                        s2 = ps_s.tile([P, NJJ, IT], F32)
                        for q in range(NJJ):
                            jc = jj * NJJ + q
                            nc.tensor.matmul(
                                s2[:, q, :],
                                KT[ds(po, KS), m, ts(jc, P)],
                                QT[ds(po, KS), m, ts(ig, IT)],
                                start=True, stop=True,
                            )
                        if jj % 2 == 0 and fillers:
                            fillers.pop(0)()
                        if prev is not None:
                            ppt, pjj = prev
                            for q in range(NJJ):
                                jc = pjj * NJJ + q
                                nc.tensor.matmul(
                                    o_ps[:], V_aug[:, h, jc, :], ppt[:, q, :],
                                    start=(jc == 0), stop=False,
                                )
                        pt = probs_pool.tile([P, NJJ, IT], MMDT)
                        nc.scalar.activation(
                            pt[:].rearrange("p a b -> p (a b)"),
                            s2[:].rearrange("p a b -> p (a b)"),
                            EXP, scale=0.125)
                        prev = (pt, jj)
                    ppt, pjj = prev
                    for q in range(NJJ):
                        jc = pjj * NJJ + q
                        nc.tensor.matmul(
                            o_ps[:], V_aug[:, h, jc, :], ppt[:, q, :],
                            start=False, stop=(q == NJJ - 1),
                        )
                    if h % 2 == 0:
                        o_pending = o_ps
                    else:
                        den_chain(ig, m, o_pending, o_ps)
                        o_pending = None

                # drain any unpumped fillers before the next i-group
                for f in fill01 + fill23:
                    f()

            # tail: Wo of the last i-group
            for itl in range(NI):
                it = 4 * (NI - 1) + itl
                for ncol in range(NO):
                    wo_unit(it, ncol)()


def _build_program():
    nc = bacc.Bacc("TRN2", target_bir_lowering=False, debug=False,
                   num_devices=NCORES)
    xT = nc.dram_tensor("xT", (D, S), MMDT, kind="ExternalInput").ap()
    wq = nc.dram_tensor("wq", (D, GW), MMDT, kind="ExternalInput").ap()
    wk = nc.dram_tensor("wk", (D, GW), MMDT, kind="ExternalInput").ap()
    wv = nc.dram_tensor("wv", (D, GW), MMDT, kind="ExternalInput").ap()
    wo = nc.dram_tensor("wo", (GW, D), MMDT, kind="ExternalInput").ap()
    bq = nc.dram_tensor("bq", (GW,), F32, kind="ExternalInput").ap()
    bk = nc.dram_tensor("bk", (GW,), F32, kind="ExternalInput").ap()
    bv = nc.dram_tensor("bv", (GW,), F32, kind="ExternalInput").ap()
    out = nc.dram_tensor("out", (S, D), MMDT, kind="ExternalOutput").ap()
    with tile.TileContext(nc) as tc:
        _mha_core(tc, out, xT, wq, wk, wv, wo, bq, bk, bv)
    nc.compile()
    return nc


_program = None


def _get_program():
    global _program
    if _program is None:
        _program = _build_program()
    return _program


def make_in_maps(x, Wq, bq, Wk, bk, Wv, bv, Wo, bo):
    in_maps = []
    f = np.float32
    bf = mybir.dt.np(MMDT)
    for c in range(NCORES):
        b, g = divmod(c, 4)
        sl = slice(g * GW, (g + 1) * GW)
        in_maps.append({
            "xT": np.ascontiguousarray(x[b].T).astype(bf),
            "wq": np.ascontiguousarray(Wq[:, sl]).astype(bf),
            "wk": np.ascontiguousarray(Wk[:, sl]).astype(bf),
            "wv": np.ascontiguousarray(Wv[:, sl]).astype(bf),
            "wo": np.ascontiguousarray(Wo[sl, :]).astype(bf),
            "bq": np.ascontiguousarray(bq[sl], dtype=f),
            "bk": np.ascontiguousarray(bk[sl], dtype=f),
            "bv": np.ascontiguousarray(bv[sl], dtype=f),
        })
    return in_maps


def run(inputs, trace=False, tmpdir=None, **kw):
    nc = _get_program()
    in_maps = make_in_maps(**inputs)
    res = run_bass_kernel_spmd(nc, in_maps, core_ids=list(range(NCORES)),
                               trace=trace, tmpdir=tmpdir, **kw)
    bo = inputs["bo"].astype(np.float32)
    parts = [np.asarray(res.results[c]["out"], dtype=np.float32)
             for c in range(NCORES)]
    y = np.stack(
        [parts[4 * b] + parts[4 * b + 1] + parts[4 * b + 2] + parts[4 * b + 3] + bo
         for b in range(B)], axis=0)
    return y.astype(np.float32), res


def kernel(**inputs):
    y, _ = run(inputs, trace=False)
    return y
